# revision 31
# baseline (speedup 1.0000x reference)
"""Local (banded, window=3) attention TRN2 kernel, v5.

Full-input contract: kernel(**inputs) takes the complete tensors
  x [8, 1024, 384], qkv_w [1152, 384], proj_w [384, 384], proj_b [384]
and returns the full output [8, 1024, 384].

Sharding: data-parallel over batch B=8 -> one batch element per NeuronCore.

Per-core algorithm (bf16 data, fp32 PSUM, feature-major [C, N] layout):
  dx[t] = x[t-1] - x[t]  (one DVE sub over all 3 contraction chunks)
  qT, vT = W_{q,v} @ x.T;  dkT = W_k @ dx.T  (PE; dk[t] = k[t-1]-k[t])
  score DIFFERENCES (softmax is shift-invariant):
    s_l - s_c = q[t].dk[t],  s_r - s_c = -q[t].dk[t+1]
    pl products on DVE, pr on Pool; head-reduction via PE matmul
    against +-scale indicators into one 2-bank PSUM tile (h = column
    half, l/r = partition groups 0/32)
  e = exp(s) on ACT (2 full-width ops); boundary cols masked (Pool)
  p_l = e_l/(1+e_l+e_r), p_r likewise: STT + reciprocal + 2 muls (DVE)
  pb = broadcast p to the 64 feature rows of its head (PE indicator,
    r-indicator negated)
  m1 = pb_l * dL[t], m2 = pb_r_neg * dL[t+1], s12 = m1 + m2  (DVE;
    dL[t] = v[t-1]-v[t] as one consolidated sub)
  y = proj_w @ (v + s12) + b: BOTH terms fed directly into the proj
    PSUM accumulation (6 matmuls per output chunk - no attn adds at
    all), bias applied during the ACT evacuation (Identity + bias AP).
"""

import numpy as np

B, N, C = 8, 1024, 384
H, HD = 6, 64
CQKV = 3 * C  # 1152
NCORES = 8
P = 128
NHALF = N // 2  # 512
KC = C // P  # 3 contraction chunks
SCALE = float(HD) ** -0.5

_cached = {}


def _build_nc():
    import contextlib

    import concourse.bacc as bacc
    import concourse.tile as tile
    from concourse import mybir

    f32 = mybir.dt.float32
    bf16 = mybir.dt.bfloat16
    AF = mybir.ActivationFunctionType
    ADD = mybir.AluOpType.add

    nc = bacc.Bacc("TRN2", target_bir_lowering=False, debug=False,
                   num_devices=NCORES)

    d_x0 = nc.dram_tensor("x0", [P, N], bf16, kind="ExternalInput").ap()
    d_x12 = nc.dram_tensor("x12", [P, 2 * N], bf16,
                           kind="ExternalInput").ap()
    d_wqk = nc.dram_tensor("wqk", [P, 3 * 768], bf16,
                           kind="ExternalInput").ap()
    d_wvp = nc.dram_tensor("wvp", [P, 3 * 768], bf16,
                           kind="ExternalInput").ap()
    d_aux = nc.dram_tensor("aux", [P, 807], bf16,
                           kind="ExternalInput").ap()
    d_biasf = nc.dram_tensor("biasf", [P, KC], f32,
                             kind="ExternalInput").ap()
    d_yT = nc.dram_tensor("yT", [C, N], bf16, kind="ExternalOutput").ap()

    with tile.TileContext(nc) as tc, contextlib.ExitStack() as ctx:
        wpool = ctx.enter_context(tc.tile_pool(name="w", bufs=1))
        qkvpool = ctx.enter_context(tc.tile_pool(name="qkv", bufs=1))
        prodpool = ctx.enter_context(tc.tile_pool(name="prod", bufs=1))
        avpool = ctx.enter_context(tc.tile_pool(name="av", bufs=1))
        epool = ctx.enter_context(tc.tile_pool(name="e", bufs=1))
        ypool = ctx.enter_context(tc.tile_pool(name="y", bufs=4))
        # PSUM (8 banks): mm 2x[128,1024] (4) + s 1x[128,1024] (2)
        #                 + pb 2x[128,512] (2)
        mmpool = ctx.enter_context(
            tc.tile_pool(name="mm", bufs=2, space="PSUM"))
        spool = ctx.enter_context(
            tc.tile_pool(name="s", bufs=1, space="PSUM"))
        pbpool = ctx.enter_context(
            tc.tile_pool(name="pb", bufs=2, space="PSUM"))

        # ---- PE warmup: junk matmuls ramp the p-state while the
        # input DMAs are still in flight (results never read)
        warm = wpool.tile([P, NHALF], bf16, name="warm")
        nc.gpsimd.memset(warm, 0.0)
        warm_ps = spool.tile([P, NHALF], f32, tag="s", name="warm_ps")
        for _ in range(14):
            nc.tensor.matmul(warm_ps, lhsT=warm[:, 0:P], rhs=warm,
                             start=True, stop=True)

        # ---- input DMAs, all on the sync queue, first-needed first ----
        wqk = wpool.tile([P, 3 * 768], bf16, name="wqk")
        nc.sync.dma_start(out=wqk[:, 0:768], in_=d_wqk[:, 0:768])
        xall = wpool.tile([P, KC, N], bf16, name="xall")
        nc.sync.dma_start(out=xall[:, 0, :], in_=d_x0)
        nc.scalar.dma_start(out=xall[:, 1, :], in_=d_x12[:, 0:N])
        nc.scalar.dma_start(out=xall[:, 2, 0:NHALF],
                          in_=d_x12[:, N:N + NHALF])
        nc.sync.dma_start(out=wqk[:, 768:1536], in_=d_wqk[:, 768:1536])
        nc.scalar.dma_start(out=xall[:, 2, NHALF:N],
                          in_=d_x12[:, N + NHALF:2 * N])
        nc.sync.dma_start(out=wqk[:, 1536:2304],
                          in_=d_wqk[:, 1536:2304])
        aux = wpool.tile([P, 807], bf16, name="aux")
        nc.sync.dma_start(out=aux[:, 0:39], in_=d_aux[:, 0:39])
        nc.sync.dma_start(out=aux[0:6, 39:807], in_=d_aux[0:6, 39:807])
        wvp = wpool.tile([P, 3 * 768], bf16, name="wvp")
        nc.gpsimd.dma_start(out=wvp, in_=d_wvp)
        biasf = wpool.tile([P, KC], f32, name="biasf")
        nc.gpsimd.dma_start(out=biasf, in_=d_biasf)

        WQK_POS = {0: 0, 3: 1, 1: 2, 4: 3, 2: 4, 5: 5}

        def w_s1(m, kc):
            # stage-1 lhsT for output chunk m (0-2 q, 3-5 k->dk, 6-8 v)
            if m < 6:
                c0 = 384 * WQK_POS[m] + P * kc
                return wqk[:, c0:c0 + P]
            c0 = 768 * kc + P * (m - 6)
            return wvp[:, c0:c0 + P]

        def w_pj(mp, kc):
            c0 = 768 * kc + 384 + P * mp
            return wvp[:, c0:c0 + P]

        def ind_s(kc, off):
            c0 = 6 * (2 * kc + off)
            return aux[:, c0:c0 + 6]

        def bias_col(m):
            return biasf[:, m:m + 1]

        def ind_b(kc, off):
            c0 = 39 + P * (2 * kc + off)
            return aux[0:6, c0:c0 + P]

        # dx[kc][t] = x[kc][t-1] - x[kc][t]
        dx = prodpool.tile([P, KC, N], bf16, name="dx")
        nc.gpsimd.memset(dx[:, :, 0:1], 0.0)
        for kc in range(2):
            nc.vector.tensor_sub(dx[:, kc, 1:N], xall[:, kc, 0:N - 1],
                                 xall[:, kc, 1:N])
        nc.vector.tensor_sub(dx[:, 2, 1:NHALF], xall[:, 2, 0:NHALF - 1],
                             xall[:, 2, 1:NHALF])
        nc.vector.tensor_sub(dx[:, 2, NHALF:N],
                             xall[:, 2, NHALF - 1:N - 1],
                             xall[:, 2, NHALF:N])

        # v chunks land in one tile; dL likewise one sub
        vall = qkvpool.tile([P, KC, N], bf16, name="vall")
        dL = avpool.tile([P, KC, N + 1], bf16, name="dL")
        nc.gpsimd.memset(dL[:, :, 0:1], 0.0)
        nc.gpsimd.memset(dL[:, :, N:N + 1], 0.0)

        # ---- stage 1: q (0-2), dk (3-5), v (6-8) ----
        qkvT = [None] * 6

        def stage1(m):
            if m < 6:
                qt = qkvpool.tile([P, N], bf16, name=f"qkvT{m}")
                dst = qt
                qkvT[m] = qt
            else:
                dst = vall[:, m - 6, :]
            ps = mmpool.tile([P, N], f32, tag="mm")
            for h in range(2):
                for kc in range(KC):
                    rhs = dx[:, kc, :] if 3 <= m < 6 else xall[:, kc, :]
                    nc.tensor.matmul(
                        ps[:, NHALF * h:NHALF * (h + 1)],
                        lhsT=w_s1(m, kc),
                        rhs=rhs[:, NHALF * h:NHALF * (h + 1)],
                        start=(kc == 0), stop=(kc == KC - 1))
                if m >= 6:
                    # per-half evac so the v-term projections can
                    # start as soon as the h0 half lands
                    nc.scalar.copy(dst[:, NHALF * h:NHALF * (h + 1)],
                                   ps[:, NHALF * h:NHALF * (h + 1)])
            if m < 6:
                nc.scalar.copy(dst, ps)

        prods = [[None, None] for _ in range(KC)]  # [kc][l, r]

        def post_k(kc):
            q = qkvT[kc]
            dk = qkvT[3 + kc]
            pl = prodpool.tile([P, N], bf16, name=f"pl{kc}")
            nc.vector.tensor_mul(pl, q, dk)
            pr = prodpool.tile([P, N], bf16, name=f"pr{kc}")
            # col N-1 left unwritten: garbage flows into e_r[N-1], which
            # is masked to 0 right after the exp
            nc.vector.tensor_mul(pr[:, 0:N - 1], q[:, 0:N - 1],
                                 dk[:, 1:N])
            prods[kc][0] = pl
            prods[kc][1] = pr

        for m in (0, 3):
            stage1(m)
        post_k(0)
        for m in (1, 4):
            stage1(m)
        post_k(1)
        for m in (2, 5):
            stage1(m)
        post_k(2)

        # ---- scores into one 2-bank PSUM tile (h = column half) ----
        s_ps = spool.tile([P, N], f32, tag="s", name="s_ps")

        def score_mm(h):
            for off in range(2):
                sub = s_ps[32 * off:32 * off + 6,
                           NHALF * h:NHALF * (h + 1)]
                for kc in range(KC):
                    nc.tensor.matmul(
                        sub, lhsT=ind_s(kc, off),
                        rhs=prods[kc][off][:, NHALF * h:NHALF * (h + 1)],
                        start=(kc == 0), stop=(kc == KC - 1))

        # ---- per-half exp + softmax + AV + fused projection helpers ----
        p_all = [None, None]  # [h] -> (pl, pr)

        def softmax(h):
            lo = NHALF * h
            hi = lo + NHALF
            el = epool.tile([6, NHALF], bf16, tag="e", bufs=8,
                            name=f"el{h}")
            er = epool.tile([6, NHALF], bf16, tag="e", bufs=8,
                            name=f"er{h}")
            with tc.high_priority():
                nc.scalar.activation(el, s_ps[0:6, lo:hi], AF.Exp)
                nc.scalar.activation(er, s_ps[32:38, lo:hi], AF.Exp)
            # boundary: no left neighbor at t=0, no right at t=N-1
            if h == 0:
                nc.gpsimd.memset(el[:, 0:1], 0.0)
            else:
                nc.gpsimd.memset(er[:, NHALF - 1:NHALF], 0.0)
            with tc.high_priority():
                den1 = epool.tile([6, NHALF], f32, tag="ef", bufs=2)
                nc.vector.scalar_tensor_tensor(den1, el, 1.0, er, ADD,
                                               ADD)
                rec = epool.tile([6, NHALF], f32, tag="ef", bufs=2)
                nc.vector.reciprocal_approx_fast(out=rec, in_=den1)
                recb = epool.tile([6, NHALF], bf16, tag="e", bufs=8)
                nc.vector.tensor_copy(recb, rec)
                pl = epool.tile([6, NHALF], bf16, tag="p", bufs=4,
                                name=f"pl{h}")
                nc.vector.tensor_mul(pl, el, recb)
                pr = epool.tile([6, NHALF], bf16, tag="p", bufs=4,
                                name=f"pr{h}")
                nc.vector.tensor_mul(pr, er, recb)
            p_all[h] = (pl, pr)

        def bcast(h):
            out = []
            for kc in range(KC):
                for off in range(2):
                    pb = pbpool.tile([P, NHALF], f32, tag="pb",
                                     name=f"pb{h}_{kc}_{off}")
                    nc.tensor.matmul(
                        pb, lhsT=ind_b(kc, off), rhs=p_all[h][off],
                        start=True, stop=True)
                    out.append(pb)
            return out

        s12s = {}

        def av(h, kc, pbl, pbr):
            lo = NHALF * h
            hi = lo + NHALF
            if not (h == 0 and kc < 2):
                # ACT evacuates to bf16 so the DVE muls run in 2x
                # mode; the first two units read PSUM directly on the
                # (then idle) DVE so AV starts before the ACT chain
                pbls = avpool.tile([P, NHALF], bf16, tag="pbs", bufs=4)
                nc.scalar.copy(pbls, pbl)
                pbl = pbls
                pbrs = avpool.tile([P, NHALF], bf16, tag="pbs", bufs=4)
                nc.scalar.copy(pbrs, pbr)
                pbr = pbrs
            m1 = avpool.tile([P, NHALF], bf16, tag="m", bufs=6)
            nc.vector.tensor_mul(m1, pbl, dL[:, kc, lo:hi])
            m2 = avpool.tile([P, NHALF], bf16, tag="m", bufs=6)
            nc.vector.tensor_mul(m2, pbr, dL[:, kc, lo + 1:hi + 1])
            # r-indicator is negated, so s12 = m1 + m2
            s12 = avpool.tile([P, NHALF], bf16, tag="s12", bufs=6)
            nc.vector.tensor_add(s12, m1, m2)
            s12s[(h, kc)] = s12

        def proj_term(yps01, yps2, kc, rhs, start, stop,
                      mp_order=(0, 1, 2)):
            for mp in mp_order:
                out = (yps01[:, NHALF * mp:NHALF * (mp + 1)] if mp < 2
                       else yps2)
                nc.tensor.matmul(out, lhsT=w_pj(mp, kc), rhs=rhs,
                                 start=start, stop=stop)

        stage1(6)
        score_mm(0)
        softmax(0)
        stage1(7)
        score_mm(1)
        softmax(1)
        # dL after the softmax chains so they don't block the DVE queue
        nc.vector.tensor_sub(dL[:, 0:2, 1:N], vall[:, 0:2, 0:N - 1],
                             vall[:, 0:2, 1:N])
        stage1(8)
        nc.vector.tensor_sub(dL[:, 2:3, 1:N], vall[:, 2:3, 0:N - 1],
                             vall[:, 2:3, 1:N])

        yps01_0 = mmpool.tile([P, N], f32, tag="mm", name="y01_0")
        yps01_1 = mmpool.tile([P, N], f32, tag="mm", name="y01_1")
        yps2b = spool.tile([P, N], f32, tag="s", name="y2b")
        yps2_0 = yps2b[:, 0:NHALF]
        yps2_1 = yps2b[:, NHALF:N]

        pbs0 = bcast(0)
        for kc in range(KC):
            proj_term(yps01_0, yps2_0, kc, vall[:, kc, 0:NHALF],
                      kc == 0, False)
        pbs1 = bcast(1)
        for kc in range(KC):
            av(0, kc, pbs0[2 * kc], pbs0[2 * kc + 1])
        for kc in range(KC):
            proj_term(yps01_1, yps2_1, kc, vall[:, kc, NHALF:N],
                      kc == 0, False)
        for kc in range(KC):
            av(1, kc, pbs1[2 * kc], pbs1[2 * kc + 1])
        for kc in range(KC):
            proj_term(yps01_0, yps2_0, kc, s12s[(0, kc)],
                      False, kc == KC - 1,
                      mp_order=(2, 1, 0) if kc == KC - 1 else (0, 1, 2))
        for kc in range(KC):
            proj_term(yps01_1, yps2_1, kc, s12s[(1, kc)],
                      False, kc == KC - 1,
                      mp_order=(2, 1, 0) if kc == KC - 1 else (0, 1, 2))
        yps = {0: (yps01_0, yps2_0), 1: (yps01_1, yps2_1)}

        for h in range(2):
            lo = NHALF * h
            hi = lo + NHALF
            yps01, yps2 = yps[h]
            for mp in ((0, 1, 2) if h == 0 else (2, 1, 0)):
                src = (yps01[:, NHALF * mp:NHALF * (mp + 1)] if mp < 2
                       else yps2)
                yt = ypool.tile([P, NHALF], bf16, tag="y")
                on_act = (h == 0 and mp == 1) or (h == 1 and mp != 1)
                if on_act:
                    nc.scalar.add(yt, src, bias_col(mp))
                else:
                    nc.vector.tensor_scalar_add(yt, src, bias_col(mp))
                nc.sync.dma_start(
                    out=d_yT[P * mp:P * (mp + 1), lo:hi], in_=yt)

    nc.compile()
    return nc


def _host_inputs(x, qkv_w, proj_w, proj_b):
    import ml_dtypes
    bf = ml_dtypes.bfloat16

    qkv_wT = np.ascontiguousarray(qkv_w.astype(np.float32).T)  # [384, 1152]
    proj_wT = np.ascontiguousarray(proj_w.astype(np.float32).T)  # [384, 384]

    # wqk packed by output chunk in PE issue order (0,3,1,4,2,5),
    # each block holding that chunk's lhsT for kc=0,1,2
    blocks = []
    for m in (0, 3, 1, 4, 2, 5):
        col = P * m if m < 3 else 384 + P * (m - 3)
        for kc in range(KC):
            blocks.append(qkv_wT[P * kc:P * (kc + 1), col:col + P])
    wqk = np.concatenate(blocks, axis=1)
    # wvp block kc = [qkv_wT v-part | proj_wT] rows of chunk kc
    wvp = np.concatenate(
        [np.concatenate([qkv_wT[P * kc:P * (kc + 1), 768:1152],
                         proj_wT[P * kc:P * (kc + 1), :]], axis=1)
         for kc in range(KC)],
        axis=1)

    aux = np.zeros((P, 807), np.float32)
    for kc in range(KC):
        for p in range(P):
            h = 2 * kc + p // HD
            # ind_s: block (kc, l) = +scale, block (kc, r) = -scale
            # (sign of the r score difference)
            aux[p, 6 * (2 * kc + 0) + h] = SCALE
            aux[p, 6 * (2 * kc + 1) + h] = -SCALE
    for m in range(KC):
        aux[:, 36 + m] = proj_b[P * m:P * (m + 1)].astype(np.float32)
    for kc in range(KC):
        for i in range(P):
            h = 2 * kc + i // HD
            aux[h, 39 + P * (2 * kc + 0) + i] = 1.0
            aux[h, 39 + P * (2 * kc + 1) + i] = -1.0

    biasf = np.stack([proj_b[P * m:P * (m + 1)].astype(np.float32)
                      for m in range(KC)], axis=1)
    shared = {
        "wqk": wqk.astype(bf),
        "wvp": wvp.astype(bf),
        "aux": aux.astype(bf),
        "biasf": biasf,
    }
    in_maps = []
    for b in range(B):
        m = dict(shared)
        xT = np.ascontiguousarray(x[b].astype(np.float32).T)  # [384, 1024]
        m["x0"] = xT[0:P, :].astype(bf)
        m["x12"] = np.concatenate([xT[P:2 * P, :], xT[2 * P:3 * P, :]],
                                  axis=1).astype(bf)
        in_maps.append(m)
    return in_maps


def kernel(x, qkv_w, proj_w, proj_b, _trace=False):
    from concourse import bass_utils

    x = np.asarray(x)
    if "nc" not in _cached:
        _cached["nc"] = _build_nc()
    nc = _cached["nc"]
    in_maps = _host_inputs(x, np.asarray(qkv_w), np.asarray(proj_w),
                           np.asarray(proj_b))
    res = bass_utils.run_bass_kernel_spmd(
        nc, in_maps, core_ids=list(range(NCORES)), trace=_trace)
    out = np.empty((B, N, C), np.float32)
    for b in range(B):
        out[b] = res.results[b]["yT"].astype(np.float32).T
    if _trace:
        _cached["last_result"] = res
    return out
